# revision 1
# baseline (speedup 1.0000x reference)
"""EqualizedModulatedConv2d (StyleGAN2) Trainium2 kernel.

Strategy: data-parallel over batch B=16 across 8 NeuronCores (2 samples/core).
Each core runs the full pipeline for its samples:
  1. style FC: esT[i,b] = elr * (lin_scale * (style @ fcW.T)[b,i] + fc_bias[i])
  2. w2T[i,o] = sum_t wT[i,o,t]^2 (from f32r-rounded weights)
  3. denomT[o,b] = sum_i w2T[i,o] * esT[i,b]^2 ; normT = 1/sqrt(denom + 1e-8)
  4. xm = x * esT (per in-channel, per sample) -> rounded to f32r
  5. conv: implicit GEMM, 9 taps x 4 iC chunks accumulated in PSUM (f32r
     matmuls, free dim 512 = 8 rows x 64 cols of the 66-wide padded image)
  6. demod: out = acc * normT during PSUM->SBUF copy, then DMA out.

Host side: pads x spatially (66x66), transposes weight to [iC, oC, 9],
fc_weight to [S, iC], style to [S, B]; gathers per-core outputs.
"""
import numpy as np

B, IC, OC, K, H, W, S = 16, 512, 512, 3, 64, 64, 512
NCORES = 8
BL = B // NCORES          # samples per core
PW = W + 2                # padded width
RT = 8                    # output rows per tile
NRT = H // RT             # row tiles
ICC = IC // 128           # in-channel chunks
OCC = OC // 128           # out-channel chunks
SC = S // 128             # style-dim chunks
ELR = (2.0 / (IC * K * K)) ** 0.5
LIN = (2.0 / S) ** 0.5

_CACHE = {}


def _build():
    import concourse.bacc as bacc
    import concourse.mybir as mybir
    import concourse.tile as tile

    f32 = mybir.dt.float32
    f32r = mybir.dt.float32r
    ALU = mybir.AluOpType

    nc = bacc.Bacc(None, target_bir_lowering=False, debug=False)
    xp = nc.dram_tensor("xp", [BL, IC, H + 2, PW], f32, kind="ExternalInput").ap()
    wt = nc.dram_tensor("wt", [IC, OC, K * K], f32, kind="ExternalInput").ap()
    fcw = nc.dram_tensor("fcw", [S, IC], f32, kind="ExternalInput").ap()
    st = nc.dram_tensor("st", [S, BL], f32, kind="ExternalInput").ap()
    fcb = nc.dram_tensor("fcb", [IC, 1], f32, kind="ExternalInput").ap()
    y = nc.dram_tensor("y", [BL, OC, H, W], f32, kind="ExternalOutput").ap()

    TX = W // 2          # 32 winograd tiles along x
    NR = 4               # winograd taps

    with tile.TileContext(nc) as tc:
        with (
            tc.tile_pool(name="up", bufs=1) as up,
            tc.tile_pool(name="wsp", bufs=3) as wsp,
            tc.tile_pool(name="fcp", bufs=1) as fcp,
            tc.tile_pool(name="sml", bufs=1) as sml,
            tc.tile_pool(name="w2t", bufs=1) as w2t,
            tc.tile_pool(name="xin", bufs=2) as xinp,
            tc.tile_pool(name="xmp", bufs=2) as xmp,
            tc.tile_pool(name="vp", bufs=8) as vp,
            tc.tile_pool(name="itp", bufs=3) as itp,
            tc.tile_pool(name="outp", bufs=2) as outp,
            tc.tile_pool(name="acc", bufs=6, space="PSUM") as accp,
            tc.tile_pool(name="pacc", bufs=2, space="PSUM") as paccp,
        ):
            # ---- fc params ----
            st_sb = fcp.tile([128, SC, BL], f32)
            nc.sync.dma_start(st_sb[:], st.rearrange("(sc p) b -> p sc b", p=128))
            fcb_sb = fcp.tile([128, ICC], f32)
            nc.sync.dma_start(fcb_sb[:], fcb.rearrange("(ic p) z -> p (ic z)", p=128))
            fcw_r = fcw.rearrange("(sc p) i -> p sc i", p=128)
            fcw_sbs = []
            for sc in range(SC):
                fcw_chunk = fcp.tile([128, IC], f32, tag=f"fcw{sc}")
                nc.scalar.dma_start(fcw_chunk[:], fcw_r[:, sc, :])
                fcw_sbs.append(fcw_chunk)

            # ---- style FC -> esT[i, b] = elr*s ----
            ebias = sml.tile([128, ICC], f32)
            nc.scalar.mul(ebias[:], fcb_sb[:], ELR)
            es_sbs, ss_sbs = [], []
            for ic in range(ICC):
                ps = paccp.tile([128, BL], f32, tag="pp")
                for sc in range(SC):
                    nc.tensor.matmul(
                        ps[:], fcw_sbs[sc][:, ic * 128:(ic + 1) * 128], st_sb[:, sc, :],
                        start=(sc == 0), stop=(sc == SC - 1),
                    )
                es_c = sml.tile([128, BL], f32, tag=f"es{ic}")
                nc.scalar.activation(
                    es_c[:], ps[:], mybir.ActivationFunctionType.Identity,
                    bias=ebias[:, ic:ic + 1], scale=ELR * LIN,
                )
                ss_c = sml.tile([128, BL], f32, tag=f"ss{ic}")
                nc.vector.tensor_mul(ss_c[:], es_c[:], es_c[:])
                es_sbs.append(es_c)
                ss_sbs.append(ss_c)

            # ---- x load + modulate + winograd input transform ----
            xp_r = xp.rearrange("b (ic p) r c -> b ic p (r c)", p=128)
            xm_cache = {}

            def load_v(b, rt):
                if (b, rt) in xm_cache:
                    return xm_cache.pop((b, rt))
                r0 = rt * RT
                vs = []
                for ic in range(ICC):
                    xin = xinp.tile([128, (RT + 2) * PW], f32, tag="xin")
                    nc.sync.dma_start(
                        xin[:], xp_r[b, ic, :, r0 * PW:(r0 + RT + 2) * PW]
                    )
                    xmt = xmp.tile([128, (RT + 2) * PW], f32, tag="xm")
                    nc.scalar.mul(xmt[:], xin[:], es_sbs[ic][:, b:b + 1])
                    xv = xmt.rearrange("p (r two k) -> p r two k", two=2, k=PW // 2)
                    d0 = xv[:, :, 0, 0:TX]
                    d1 = xv[:, :, 1, 0:TX]
                    d2 = xv[:, :, 0, 1:TX + 1]
                    d3 = xv[:, :, 1, 1:TX + 1]
                    vt = vp.tile([128, NR, RT + 2, TX], f32r, tag="v")
                    nc.vector.tensor_sub(vt[:, 0], d0, d2)
                    nc.vector.tensor_add(vt[:, 1], d1, d2)
                    nc.vector.tensor_sub(vt[:, 2], d2, d1)
                    nc.vector.tensor_sub(vt[:, 3], d1, d3)
                    vs.append(vt)
                return vs

            # ---- weights: stream chunks, build winograd taps u + w2 ----
            wt_r = wt.rearrange("(ic p) o t -> p ic o t", p=128)
            u_sbs = []
            for ic in range(ICC):
                u_chunk = up.tile([128, OC, K, NR], f32r, tag=f"u{ic}")
                u_sbs.append(u_chunk)
            w2_sbs = {}
            for ic in range(ICC):
                for oc in range(OCC):
                    w2s = sml.tile([128, 128], f32, tag=f"w2_{ic}_{oc}")
                    w2_sbs[(ic, oc)] = w2s

            def load_wt(ic, oc):
                sl = slice(oc * 128, (oc + 1) * 128)
                ws = wsp.tile([128, 128, K, K], f32, tag="ws")
                nc.sync.dma_start(
                    ws.rearrange("p o a b -> p (o a b)"),
                    wt_r[:, ic, sl, :].rearrange("p o t -> p (o t)"),
                )
                # w2 slice for demod norm
                sq = w2t.tile([128, 128, K * K], f32, tag="w2tmp")
                wv = ws.rearrange("p o a b -> p o (a b)")
                nc.scalar.square(sq[:], wv)
                nc.vector.reduce_sum(w2_sbs[(ic, oc)][:], sq[:],
                                     axis=mybir.AxisListType.X)
                # winograd taps: u0=w0, u1=(w0+w1+w2)/2, u2=(w0-w1+w2)/2, u3=w2
                u = u_sbs[ic]
                w0, w1, w2_ = ws[:, :, :, 0], ws[:, :, :, 1], ws[:, :, :, 2]
                nc.gpsimd.tensor_copy(u[:, sl, :, 0], w0)
                nc.gpsimd.tensor_copy(u[:, sl, :, 3], w2_)
                s02 = w2t.tile([128, 128, K], f32, tag="s02")
                nc.gpsimd.tensor_add(s02[:], w0, w2_)
                w1h = w2t.tile([128, 128, K], f32, tag="w1h")
                nc.scalar.mul(w1h[:], w1, 0.5)
                nc.vector.scalar_tensor_tensor(
                    u[:, sl, :, 1], s02[:], 0.5, w1h[:], ALU.mult, ALU.add)
                nc.vector.scalar_tensor_tensor(
                    u[:, sl, :, 2], s02[:], 0.5, w1h[:], ALU.mult, ALU.subtract)

            load_wt(0, 0)
            xm_cache[(0, 0)] = load_v(0, 0)
            for ic in range(1, ICC):
                load_wt(ic, 0)
            xm_cache[(0, 1)] = load_v(0, 1)
            for oc in range(1, OCC):
                for ic in range(ICC):
                    load_wt(ic, oc)

            # ---- demod norm: normT[o, b] (per-oc as w2 slices land) ----
            norm_sb = sml.tile([128, OCC, BL], f32)
            sqd = sml.tile([128, OCC, BL], f32)
            eps_sb = sml.tile([128, 1], f32)
            nc.vector.memset(eps_sb[:], 1e-8)
            for oc in range(OCC):
                pd = paccp.tile([128, BL], f32, tag="pp")
                for ic in range(ICC):
                    nc.tensor.matmul(
                        pd[:], w2_sbs[(ic, oc)][:], ss_sbs[ic][:],
                        start=(ic == 0), stop=(ic == ICC - 1),
                    )
                nc.scalar.activation(
                    sqd[:, oc, :], pd[:], mybir.ActivationFunctionType.Sqrt,
                    bias=eps_sb[:],
                )
                nc.vector.reciprocal(norm_sb[:, oc, :], sqd[:, oc, :])

            # ---- main winograd-conv loop ----
            def conv_group(b, rt, vs, oc):
                    r0 = rt * RT
                    if True:
                        osl = slice(oc * 128, (oc + 1) * 128)
                        psA = accp.tile([128, 2, RT * TX], f32, tag="wacc")
                        psB = accp.tile([128, 2, RT * TX], f32, tag="wacc")
                        for r in range(NR):
                            ps = psA if r < 2 else psB
                            j = r % 2
                            for ic in range(ICC):
                                for dy in range(K):
                                    nc.tensor.matmul(
                                        ps[:, j, :],
                                        u_sbs[ic][:, osl, dy, r],
                                        vs[ic][:, r, dy:dy + RT, :],
                                        start=(ic == 0 and dy == 0),
                                        stop=(ic == ICC - 1 and dy == K - 1),
                                    )
                        # inverse transform + demod + store
                        m0, m1 = psA[:, 0, :], psA[:, 1, :]
                        m2, m3 = psB[:, 0, :], psB[:, 1, :]
                        nv = norm_sb[:, oc, b:b + 1]
                        c1 = itp.tile([128, RT * TX], f32, tag="it")
                        nc.scalar.copy(c1[:], m1)
                        a01 = itp.tile([128, RT * TX], f32, tag="it")
                        nc.vector.tensor_add(a01[:], c1[:], m0)
                        t012 = itp.tile([128, RT * TX], f32, tag="it")
                        nc.vector.tensor_add(t012[:], a01[:], m2)
                        b13 = itp.tile([128, RT * TX], f32, tag="it")
                        nc.vector.tensor_sub(b13[:], c1[:], m3)
                        t123 = itp.tile([128, RT * TX], f32, tag="it")
                        nc.vector.tensor_sub(t123[:], b13[:], m2)
                        ot = outp.tile([128, RT * W], f32, tag="ot")
                        ov = ot.rearrange("p (r k two) -> p r k two", two=2, k=TX)
                        tv0 = t012.rearrange("p (r k) -> p r k", k=TX)
                        tv1 = t123.rearrange("p (r k) -> p r k", k=TX)
                        nc.scalar.mul(ov[:, :, :, 0], tv0, nv)
                        nc.scalar.mul(ov[:, :, :, 1], tv1, nv)
                        nc.sync.dma_start(
                            y[b, osl, r0:r0 + RT, :].rearrange("p r c -> p (r c)"),
                            ot[:],
                        )

            # first two row-tiles of b0 interleaved oc-outer: each arriving
            # weight column-chunk enables 2 groups of PE work during the
            # initial weight stream
            vs00 = load_v(0, 0)
            vs01 = load_v(0, 1)
            for oc in range(2):
                conv_group(0, 0, vs00, oc)
                conv_group(0, 1, vs01, oc)
            conv_group(0, 0, vs00, 2)
            conv_group(0, 0, vs00, 3)
            conv_group(0, 1, vs01, 2)
            conv_group(0, 1, vs01, 3)
            for b in range(BL):
                for rt in range(NRT):
                    if b == 0 and rt < 2:
                        continue
                    vs = load_v(b, rt)
                    for oc in range(OCC):
                        conv_group(b, rt, vs, oc)
    nc.compile()
    return nc


class _Runner:
    """Persistent jitted PJRT executor for the SPMD kernel (axon path)."""

    def __init__(self, nc, n_cores):
        import jax
        import numpy as np
        from jax.sharding import Mesh, PartitionSpec
        try:
            from jax.experimental.shard_map import shard_map
        except ImportError:
            from jax.shard_map import shard_map
        import concourse.mybir as mybir
        from concourse.bass2jax import (
            _bass_exec_p, install_neuronx_cc_hook, partition_id_tensor,
        )

        install_neuronx_cc_hook()
        self.jax = jax
        self.n_cores = n_cores
        partition_name = (
            nc.partition_id_tensor.name if nc.partition_id_tensor else None
        )
        in_names, out_names, out_avals, zero_outs = [], [], [], []
        for alloc in nc.m.functions[0].allocations:
            if not isinstance(alloc, mybir.MemoryLocationSet):
                continue
            name = alloc.memorylocations[0].name
            if alloc.kind == "ExternalInput":
                if name != partition_name:
                    in_names.append(name)
            elif alloc.kind == "ExternalOutput":
                out_names.append(name)
                shape = tuple(alloc.tensor_shape)
                dtype = mybir.dt.np(alloc.dtype)
                out_avals.append(jax.core.ShapedArray(shape, dtype))
                zero_outs.append(np.zeros(shape, dtype))
        self.in_names, self.out_names, self.out_avals = in_names, out_names, out_avals

        def _body(*args):
            operands = list(args)
            if partition_name is not None:
                operands.append(partition_id_tensor())
            return tuple(
                _bass_exec_p.bind(
                    *operands,
                    out_avals=tuple(out_avals),
                    in_names=tuple(in_names + out_names + ([partition_name] if partition_name else [])),
                    out_names=tuple(out_names),
                    lowering_input_output_aliases=(),
                    sim_require_finite=False,
                    sim_require_nnan=False,
                    nc=nc,
                )
            )

        devices = jax.devices()[:n_cores]
        mesh = Mesh(np.asarray(devices), ("core",))
        n_params = len(in_names)
        self.fn = jax.jit(
            shard_map(
                _body, mesh=mesh,
                in_specs=(PartitionSpec("core"),) * (n_params + len(out_names)),
                out_specs=(PartitionSpec("core"),) * len(out_names),
                check_rep=False,
            ),
            keep_unused=True,
        )
        self.sharding = jax.sharding.NamedSharding(mesh, PartitionSpec("core"))
        self._dev_zeros = [
            jax.device_put(
                np.zeros((n_cores * z.shape[0], *z.shape[1:]), z.dtype), self.sharding
            )
            for z in zero_outs
        ]

    def put_inputs(self, in_maps):
        concat = [
            np.concatenate(
                [np.asarray(in_maps[c][n]) for c in range(self.n_cores)], axis=0
            )
            for n in self.in_names
        ]
        return [self.jax.device_put(a, self.sharding) for a in concat]

    def run(self, dev_args):
        outs = self.fn(*dev_args, *self._dev_zeros)
        self.jax.block_until_ready(outs)
        return outs

    def results(self, outs):
        res = []
        for c in range(self.n_cores):
            d = {}
            for i, name in enumerate(self.out_names):
                full = np.asarray(outs[i])
                d[name] = full.reshape(self.n_cores, *self.out_avals[i].shape)[c]
            res.append(d)
        return res


def _get_runner():
    if "runner" not in _CACHE:
        nc = _build()
        _CACHE["nc"] = nc
        _CACHE["runner"] = _Runner(nc, NCORES)
    return _CACHE["runner"]


def _prep_inputs(x, style, weight, fc_weight, fc_bias):
    """Host-side sharding + layout marshalling. Returns per-core input maps."""
    x = np.asarray(x, dtype=np.float32)
    style = np.asarray(style, dtype=np.float32)
    weight = np.asarray(weight, dtype=np.float32)
    fc_weight = np.asarray(fc_weight, dtype=np.float32)
    fc_bias = np.asarray(fc_bias, dtype=np.float32)

    xpad = np.zeros((B, IC, H + 2, PW), dtype=np.float32)
    xpad[:, :, 1:H + 1, 1:W + 1] = x
    # de-interleave columns: row layout [even cols | odd cols] so the
    # winograd input-transform reads contiguous runs
    xpad = np.ascontiguousarray(
        xpad.reshape(B, IC, H + 2, PW // 2, 2).transpose(0, 1, 2, 4, 3)
    ).reshape(B, IC, H + 2, PW)
    wt_host = np.ascontiguousarray(
        weight.transpose(1, 0, 2, 3).reshape(IC, OC, K * K)
    )
    fcw_host = np.ascontiguousarray(fc_weight.T)
    fcb_host = np.ascontiguousarray(fc_bias.reshape(IC, 1))

    in_maps = []
    for c in range(NCORES):
        sl = slice(c * BL, (c + 1) * BL)
        in_maps.append({
            "xp": np.ascontiguousarray(xpad[sl]),
            "wt": wt_host,
            "fcw": fcw_host,
            "st": np.ascontiguousarray(style[sl].T),
            "fcb": fcb_host,
        })
    return in_maps


def kernel(x, style, weight, fc_weight, fc_bias):
    runner = _get_runner()
    in_maps = _prep_inputs(x, style, weight, fc_weight, fc_bias)
    dev_args = runner.put_inputs(in_maps)
    outs = runner.run(dev_args)
    res = runner.results(outs)
    out = np.concatenate([res[c]["y"] for c in range(NCORES)], axis=0)
    return out.astype(np.float32)



# revision 66
# speedup vs baseline: 1.2237x; 1.2237x over previous
"""EqualizedModulatedConv2d (StyleGAN2) Trainium2 kernel.

Data-parallel over batch B=16 across 8 NeuronCores (2 samples/core).
Conv uses 1-D Winograd F(4,3) along the width axis, all-fp16 matmuls
(1 cycle/row on the PE at free>=128) with fp32 PSUM accumulation:
  out cols 4w..4w+3 come from 6 taps V_j (x-transform) x U_j (w-transform),
  summed over (ic, dy) in 6 PSUM chains of 12 matmuls, free dim 256.
Engine balance (TimelineSim-guided):
  PE   : conv chains + style-FC + norm denominators (w^2 9-way sum folded
         into the norm matmul chain over per-(ic,oc) squared weights)
  Act  : modulate xm=es*x and xm4=4*xm (fp16), demod-folded PSUM
         evacuation (per-tap scales 1/4,-1/6,1/6,1/48,1/48,1 baked into
         per-(oc,b) scalars), weight squares
  DVE  : input transform as 15 2x-speed fp16 add/subs on xm/xm4 (host
         mod-4 column de-interleave keeps every slice stride-1),
         inverse-transform finals, stt halves of weight prep
  Pool : adds half of weight prep, inverse-transform pre-combines,
         weight/output DMA queue (25ns issues)
Scheduling: emission order hand-pipelined (norm recip must precede the
first dependent evac in every engine FIFO to avoid deadlock); PE warm-up
chains hold the p-state ramp (idle gaps reset the PE clock to 1.2GHz for
3us); extras (weight prep / squares / norms) spread one ic per group
boundary; final group split into 8-row halves to shrink the drain tail.
Host side: pad x to 66x66, group columns by (c mod 4), cast x/w/fc to fp16.
"""
import numpy as np

B, IC, OC, K, H, W, S = 16, 512, 512, 3, 64, 64, 512
NCORES = 8
BL = B // NCORES          # samples per core
PR, PC = H + 2, W + 2     # padded rows/cols
NW = W // 4               # 16 winograd windows along x
ICC = IC // 128
OCC = OC // 128
SC = S // 128
ELR = (2.0 / (IC * K * K)) ** 0.5
LIN = (2.0 / S) ** 0.5
NWIN = 2                  # two 34-row window bands per sample
WROWS = 34

_CACHE = {}
_DEBUG = False
WARMUP_CHAINS = 30


def _build():
    import concourse.bacc as bacc
    import concourse.mybir as mybir
    import concourse.tile as tile

    f32 = mybir.dt.float32
    f16 = mybir.dt.float16
    ALU = mybir.AluOpType
    AF = mybir.ActivationFunctionType

    nc = bacc.Bacc(None, target_bir_lowering=False, debug=False)
    xp = nc.dram_tensor("xp", [BL, IC, PR * PC], f16, kind="ExternalInput").ap()
    wt = nc.dram_tensor("wt", [IC, K, K, OC], f16, kind="ExternalInput").ap()
    fcw = nc.dram_tensor("fcw", [S, IC], f16, kind="ExternalInput").ap()
    st = nc.dram_tensor("st", [S, BL], f16, kind="ExternalInput").ap()
    fcb = nc.dram_tensor("fcb", [IC, 1], f32, kind="ExternalInput").ap()
    y = nc.dram_tensor("y", [BL, OC, H, W], f32, kind="ExternalOutput").ap()
    if _DEBUG:
        d_es = nc.dram_tensor("d_es", [IC, BL], f32, kind="ExternalOutput").ap()
        d_norm = nc.dram_tensor("d_norm", [OC, BL], f32, kind="ExternalOutput").ap()
        d_w2 = nc.dram_tensor("d_w2", [IC, OC], f16, kind="ExternalOutput").ap()
        d_v = nc.dram_tensor("d_v", [IC, 6, WROWS, NW], f16, kind="ExternalOutput").ap()
        d_m = nc.dram_tensor("d_m", [128, 6, 256], f16, kind="ExternalOutput").ap()

    xp_r = xp.rearrange("b (ic p) rc -> b ic p rc", p=128)
    wt_r = wt.rearrange("(ic p) k d o -> p ic k d o", p=128)

    with tile.TileContext(nc) as tc:
        with (
            tc.tile_pool(name="wp", bufs=1) as wp,
            tc.tile_pool(name="w1p", bufs=4) as w1p,
            tc.tile_pool(name="wtmp", bufs=2) as wtmp,
            tc.tile_pool(name="fcp", bufs=1) as fcp,
            tc.tile_pool(name="w2p", bufs=4) as w2p,
            tc.tile_pool(name="vp", bufs=10) as vp,
            tc.tile_pool(name="xinp", bufs=1) as xinp,
            tc.tile_pool(name="xmp", bufs=2) as xmp,
            tc.tile_pool(name="xtp", bufs=1) as xtp,
            tc.tile_pool(name="mp", bufs=2) as mp,
            tc.tile_pool(name="itp", bufs=1) as itp,
            tc.tile_pool(name="otp", bufs=2) as otp,
            tc.tile_pool(name="acc", bufs=7, space="PSUM") as accp,
            tc.tile_pool(name="pacc", bufs=1, space="PSUM") as paccp,
        ):
            # ---- fc param DMAs (sync queue, tiny, first) ----
            st_sb = fcp.tile([128, SC, BL], f16)
            nc.sync.dma_start(st_sb[:], st.rearrange("(sc p) b -> p sc b", p=128))
            fcb_sb = fcp.tile([128, ICC], f32)
            nc.sync.dma_start(fcb_sb[:], fcb.rearrange("(ic p) z -> p (ic z)", p=128))
            fcw_sb = fcp.tile([128, SC, IC], f16)
            nc.sync.dma_start(fcw_sb[:], fcw.rearrange("(sc p) i -> p sc i", p=128))

            # ---- weight DMAs (Pool queue: 25ns issues) ----
            w02_sbs, w1_sbs = [], []
            for ic in range(ICC):
                w02 = wp.tile([128, 2, K, OC], f16, tag=f"w02_{ic}")
                nc.gpsimd.dma_start(w02[:], wt_r[:, ic, 0::2, :, :])
                w1s = w1p.tile([128, K, OC], f16, tag="w1")
                nc.gpsimd.dma_start(w1s[:], wt_r[:, ic, 1, :, :])
                w02_sbs.append(w02)
                w1_sbs.append(w1s)

            # ---- style FC: es[i,b] (includes elr), f32 ----
            ebias = fcp.tile([128, ICC], f32)
            nc.scalar.mul(ebias[:], fcb_sb[:], ELR)
            eps_sb = fcp.tile([128, 1], f32)
            nc.vector.memset(eps_sb[:], 1e-8)
            es_sbs, ss_sbs = [], []
            for ic in range(ICC):
                ps = paccp.tile([128, BL], f32, tag="pp")
                for sc in range(SC):
                    nc.tensor.matmul(
                        ps[:], fcw_sb[:, sc, ic * 128:(ic + 1) * 128],
                        st_sb[:, sc, :],
                        start=(sc == 0), stop=(sc == SC - 1),
                    )
                es_c = fcp.tile([128, BL], f32, tag=f"es{ic}")
                nc.scalar.activation(
                    es_c[:], ps[:], AF.Identity,
                    bias=ebias[:, ic:ic + 1], scale=ELR * LIN,
                )
                ss_c = fcp.tile([128, BL], f16, tag=f"ss{ic}")
                nc.vector.tensor_mul(ss_c[:], es_c[:], es_c[:])
                es_sbs.append(es_c)
                ss_sbs.append(ss_c)
                if _DEBUG:
                    nc.sync.dma_start(d_es[ic * 128:(ic + 1) * 128, :], es_c[:])

            # ---- weight tap prep per (ic, oc), DVE only, UNSCALED taps:
            #   U1 = w0+w1+w2, U2 = w1-(w0+w2), U3 = w0+2w1+4w2, U4 = w0-2w1+4w2
            # the 1/6 and 1/24 factors are folded into the demod scalars.
            uu_sbs = {}
            for ic in range(ICC):
                for oc in range(OCC):
                    uu_sbs[(ic, oc)] = wp.tile(
                        [128, 4, K, 128], f16, tag=f"uu{ic}_{oc}",
                        name=f"uu{ic}_{oc}")

            def prep_w_pool(ic, oc):
                # Pool chain: a, U1, U2 (adds)
                sl = slice(oc * 128, (oc + 1) * 128)
                w0 = w02_sbs[ic][:, 0, :, sl]    # [128, 3, 128]
                w2_ = w02_sbs[ic][:, 1, :, sl]
                w1 = w1_sbs[ic][:, :, sl]
                uu = uu_sbs[(ic, oc)]
                atile = wtmp.tile([128, K, 128], f16, tag="wtmpa")
                nc.gpsimd.tensor_add(atile[:], w0, w2_)
                nc.gpsimd.tensor_add(uu[:, 0], atile[:], w1)
                nc.gpsimd.tensor_sub(uu[:, 1], w1, atile[:])

            def prep_w_dve(ic, oc):
                # DVE chain: c, U3, U4 (stt)
                sl = slice(oc * 128, (oc + 1) * 128)
                w0 = w02_sbs[ic][:, 0, :, sl]
                w2_ = w02_sbs[ic][:, 1, :, sl]
                w1 = w1_sbs[ic][:, :, sl]
                uu = uu_sbs[(ic, oc)]
                ctile = wtmp.tile([128, K, 128], f16, tag="wtmpc")
                nc.vector.scalar_tensor_tensor(
                    ctile[:], w2_, 4.0, w0, ALU.mult, ALU.add)
                nc.vector.scalar_tensor_tensor(
                    uu[:, 2], w1, 2.0, ctile[:], ALU.mult, ALU.add)
                nc.vector.scalar_tensor_tensor(
                    uu[:, 3], w1, -2.0, ctile[:], ALU.mult, ALU.add)

            def prep_w(ic, oc):
                prep_w_pool(ic, oc)
                prep_w_dve(ic, oc)

            # ---- w2 (demod) per (ic, oc): Act square; the 9-way sum is
            # folded into the norm matmul chain (PE contracts ic, loops k).
            sq_tiles = {}

            def w2_square(ic, oc, pool=False, dve=False):
                sl = slice(oc * 128, (oc + 1) * 128)
                sq = w2p.tile([128, 3 * K, 128], f16, tag="sq")
                src_w02 = w02_sbs[ic][:, :, :, sl].rearrange("p a d o -> p (a d) o")
                src_w1 = w1_sbs[ic][:, :, sl]
                if pool:
                    nc.gpsimd.tensor_mul(sq[:, 0:6, :], src_w02, src_w02)
                    nc.gpsimd.tensor_mul(sq[:, 6:9, :], src_w1, src_w1)
                elif dve:
                    nc.vector.tensor_mul(sq[:, 0:6, :], src_w02, src_w02)
                    nc.vector.tensor_mul(sq[:, 6:9, :], src_w1, src_w1)
                else:
                    nc.scalar.square(sq[:, 0:6, :], src_w02)
                    nc.scalar.square(sq[:, 6:9, :], src_w1)
                sq_tiles[(ic, oc)] = sq

            # ---- norm per oc plus tap-scale variants (1/4, -1/6, 1/6, 1/48) ----
            norm_sb = fcp.tile([128, OCC, BL], f32)
            norm4_sb = fcp.tile([128, OCC, BL], f32)
            norm6n_sb = fcp.tile([128, OCC, BL], f32)
            norm6_sb = fcp.tile([128, OCC, BL], f32)
            norm48_sb = fcp.tile([128, OCC, BL], f32)
            sqd = fcp.tile([128, OCC, BL], f32)

            def norm_oc(oc):
                pd = paccp.tile([128, BL], f32, tag="pp")
                for ic in range(ICC):
                    sq = sq_tiles.pop((ic, oc))
                    for k in range(3 * K):
                        nc.tensor.matmul(
                            pd[:], sq[:, k, :], ss_sbs[ic][:],
                            start=(ic == 0 and k == 0),
                            stop=(ic == ICC - 1 and k == 3 * K - 1),
                        )
                nc.scalar.activation(sqd[:, oc, :], pd[:], AF.Sqrt, bias=eps_sb[:])
                nc.vector.reciprocal(norm_sb[:, oc, :], sqd[:, oc, :])
                nc.scalar.mul(norm4_sb[:, oc, :], norm_sb[:, oc, :], 1.0 / 4.0)
                nc.scalar.mul(norm6n_sb[:, oc, :], norm_sb[:, oc, :], -1.0 / 6.0)
                nc.scalar.mul(norm6_sb[:, oc, :], norm_sb[:, oc, :], 1.0 / 6.0)
                nc.scalar.mul(norm48_sb[:, oc, :], norm_sb[:, oc, :], 1.0 / 48.0)
                if _DEBUG:
                    nc.sync.dma_start(d_norm[oc * 128:(oc + 1) * 128, :],
                                      norm_sb[:, oc, :])

            # ---- modulate + input transform per (b, win, ic) ----
            vt_tiles = {}

            def load_x(b, win, ic):
                r0p = 32 * win
                xin = xinp.tile([128, WROWS * PC], f16, tag="xin")
                nc.sync.dma_start(
                    xin[:], xp_r[b, ic, :, r0p * PC:(r0p + WROWS) * PC])
                return xin

            def modulate(b, ic, xin):
                xm = xmp.tile([128, WROWS * PC], f16, tag="xm")
                nc.scalar.mul(xm[:], xin[:], es_sbs[ic][:, b:b + 1])
                xm4 = xmp.tile([128, WROWS * PC], f16, tag="xm4")
                nc.scalar.mul(xm4[:], xm[:], 4.0)
                return xm, xm4

            def transform(b, win, ic, xms):
                """Input transform: all 2x-speed DVE adds/subs using xm and
                4*xm; tap scales folded into the demod scalars."""
                xm, xm4 = xms
                xv = xm.rearrange("p (r c) -> p r c", c=PC)
                x4 = xm4.rearrange("p (r c) -> p r c", c=PC)
                d0q, d1q, d2q, d3q = (x4[:, :, 0:16], x4[:, :, 17:33],
                                      x4[:, :, 34:50], x4[:, :, 50:66])
                d1 = xv[:, :, 17:33]
                d5 = xv[:, :, 18:34]
                d2 = xv[:, :, 34:50]
                d3 = xv[:, :, 50:66]
                d4 = xv[:, :, 1:17]
                sc3 = xtp.tile([128, 3, WROWS, NW], f16, tag="sc")
                s0, s1, s2 = (sc3[:, i] for i in range(3))
                vt = vp.tile([128, 6, WROWS, NW], f16, tag="v")
                nc.vector.tensor_sub(s0, d4, d2)          # t
                nc.vector.tensor_add(s1, s0, s0)          # t2
                nc.vector.tensor_sub(s2, d0q, d2q)        # u4t
                nc.vector.tensor_add(vt[:, 0], s2, s0)    # v0 = u4t + t
                nc.vector.tensor_sub(s0, d1q, d3q)        # r (t dead)
                nc.vector.tensor_sub(vt[:, 3], s1, s0)    # v3 = t2 - r
                nc.vector.tensor_add(vt[:, 4], s1, s0)    # v4 = t2 + r
                nc.vector.tensor_sub(s1, d5, d3)          # w (t2 dead)
                nc.vector.tensor_add(vt[:, 5], s0, s1)    # v5 = r + w
                nc.vector.tensor_add(s0, d3, d4)          # a1 (r dead)
                nc.vector.tensor_add(s1, d1q, d2q)        # a2
                nc.vector.tensor_sub(vt[:, 1], s0, s1)    # v1 = a1 - a2
                nc.vector.tensor_sub(s0, d4, d3)          # b1
                nc.vector.tensor_sub(s1, d1q, d2q)        # b2
                nc.vector.tensor_add(vt[:, 2], s1, s0)    # v2 = b2 + b1
                vt_tiles[(b, win, ic)] = vt
                if _DEBUG and b == 0 and win == 0:
                    nc.sync.dma_start(
                        d_v[ic * 128:(ic + 1) * 128].rearrange(
                            "p j r w -> p (j r w)"),
                        vt.rearrange("p j r w -> p (j r w)"))

            def full_transform(b, win):
                xins = [load_x(b, win, ic) for ic in range(ICC)]
                for ic in range(ICC):
                    xm = modulate(b, ic, xins[ic])
                    transform(b, win, ic, xm)

            # ---- conv group ----
            vt_off = {}

            def conv_group(b, win, oc, lr, nr=16, sub=0):
                sl = slice(oc * 128, (oc + 1) * 128)
                fr = nr * NW
                pts = [accp.tile([128, 2, 256], f32, tag="acc", name=f"acc{i}")
                       for i in range(3)]
                # j-major: each PSUM accumulation region runs start..stop
                # contiguously (interleaving regions within a bank corrupts
                # the accumulation). Chain order by operand readiness at
                # startup: ws taps first, then Pool-prepped, then DVE-prepped.
                r0v = (16 + vt_off.get((b, win), 0)) * lr + 8 * sub
                for j in (0, 5, 1, 2, 3, 4):
                    pt = pts[j // 2][:, j % 2, 0:fr]
                    for ic in range(ICC):
                        if j == 0:
                            statT = w02_sbs[ic][:, 0, :, sl]
                        elif j == 5:
                            statT = w02_sbs[ic][:, 1, :, sl]
                        else:
                            statT = uu_sbs[(ic, oc)][:, j - 1]
                        vt = vt_tiles[(b, win, ic)]
                        for dy in range(K):
                            nc.tensor.matmul(
                                pt, statT[:, dy],
                                vt[:, j, r0v + dy:r0v + dy + nr, :],
                                start=(ic == 0 and dy == 0),
                                stop=(ic == ICC - 1 and dy == K - 1),
                            )
                # demod-folded evacuation (Act); the tap scales (1/4, -1/6,
                # 1/6, 1/48, 1/48, 1) fold into the per-(oc,b) scalar here.
                nvs = [norm4_sb, norm6n_sb, norm6_sb, norm48_sb, norm48_sb, norm_sb]
                m_sb = mp.tile([128, 6, 256], f16, tag="m")
                for j in range(6):
                    nc.scalar.mul(m_sb[:, j, 0:fr], pts[j // 2][:, j % 2, 0:fr],
                                  nvs[j][:, oc, b:b + 1])
                # inverse transform (pre-combines on Pool, rest on DVE)
                it = itp.tile([128, 6, 256], f16, tag="it")
                s12, d12, s34, d34, t0, t3 = (it[:, i, 0:fr] for i in range(6))
                m = [m_sb[:, j, 0:fr] for j in range(6)]
                nc.gpsimd.tensor_add(s12, m[1], m[2])
                nc.gpsimd.tensor_sub(d12, m[1], m[2])
                nc.gpsimd.tensor_add(s34, m[3], m[4])
                nc.gpsimd.tensor_sub(d34, m[3], m[4])
                ot = otp.tile([128, 16, W], f32, tag="ot")
                ov = ot.rearrange("p r (w four) -> p four r w", four=4)[:, :, 0:nr, :]

                def rs(ap):
                    return ap.rearrange("p (r w) -> p r w", w=NW)

                nc.vector.tensor_add(t0, m[0], s12)
                nc.vector.scalar_tensor_tensor(
                    ov[:, 1], rs(d34), 2.0, rs(d12), ALU.mult, ALU.add)
                nc.vector.scalar_tensor_tensor(
                    ov[:, 2], rs(s34), 4.0, rs(s12), ALU.mult, ALU.add)
                nc.vector.scalar_tensor_tensor(
                    t3, d34, 8.0, d12, ALU.mult, ALU.add)
                nc.vector.tensor_add(ov[:, 0], rs(t0), rs(s34))
                nc.vector.tensor_add(ov[:, 3], rs(t3), rs(m[5]))
                # store (Pool queue: cheap issue)
                r0 = 32 * win + 16 * lr + 8 * sub
                nc.gpsimd.dma_start(
                    y[b, sl, r0:r0 + nr, :].rearrange("p r c -> p (r c)"),
                    ot.rearrange("p r w -> p (r w)")[:, 0:nr * W],
                )

            # ================= emission schedule =================
            # startup: x-pipeline for (b0,w0); transforms first on DVE, then
            # weight preps, then the oc0 w2 path.
            for ic in range(ICC):
                w2_square(ic, 0, pool=True)
            for ic in range(ICC):
                xin = load_x(0, 0, ic)
                xm = modulate(0, ic, xin)
                transform(0, 0, ic, xm)
            for ic in range(ICC):
                prep_w(ic, 0)
            for ic in range(ICC):
                prep_w_pool(ic, 1)
            # PE warm-up: dummy chains keep the p-state ramp hot while V/U
            # stream in (gaps reset the PE clock to 1.2GHz for 3us).
            for c in range(WARMUP_CHAINS):
                wu = accp.tile([128, 2, 256], f32, tag="acc", name=f"wu{c}")
                for k in range(4):
                    nc.tensor.matmul(
                        wu[:, 0], fcw_sb[:, k, 0:128], fcw_sb[:, k, 0:256],
                        start=(k == 0), stop=(k == 3),
                    )
            # norm oc0 must precede the first oc0 group in Act/DVE FIFOs
            # (evac waits on norm -> recip must come before group inverse).
            norm_oc(0)
            # b0: oc-major pairs (w0 pair then w1 pair); extras for oc k+1
            # spread one ic per group boundary across oc k's four groups.
            def extras(ic, oc):
                w2_square(ic, oc)
                if oc == 1 or oc == 3:
                    prep_w_dve(ic, oc)
                else:
                    prep_w(ic, oc)

            def xpipe(b, win, ic):
                transform(b, win, ic, modulate(b, ic, load_x(b, win, ic)))

            # b0 order: w0-oc0, w0-oc1, w1-oc0, w1-oc1, w0-oc2, w0-oc3,
            # w1-oc2, w1-oc3 — extras and transforms spread so each engine's
            # per-phase load stays under the PE group rate.
            conv_group(0, 0, 0, 0)              # 1
            extras(0, 1)
            extras(1, 1)
            xpipe(0, 1, 0)
            conv_group(0, 0, 0, 1)              # 2
            extras(2, 1)
            extras(3, 1)
            norm_oc(1)
            xpipe(0, 1, 1)
            conv_group(0, 0, 1, 0)              # 3
            xpipe(0, 1, 2)
            conv_group(0, 0, 1, 1)              # 4
            xpipe(0, 1, 3)
            conv_group(0, 1, 0, 0)              # 5
            extras(0, 2)
            conv_group(0, 1, 0, 1)              # 6
            extras(1, 2)
            conv_group(0, 1, 1, 0)              # 7
            extras(2, 2)
            prep_w_pool(0, 3)
            prep_w_pool(1, 3)
            extras(0, 3)
            conv_group(0, 1, 1, 1)              # 8
            extras(3, 2)
            prep_w_pool(2, 3)
            prep_w_pool(3, 3)
            extras(1, 3)
            norm_oc(2)
            conv_group(0, 0, 2, 0)              # 9
            extras(2, 3)
            conv_group(0, 0, 2, 1)              # 10
            extras(3, 3)
            norm_oc(3)
            conv_group(0, 0, 3, 0)              # 11
            conv_group(0, 0, 3, 1)              # 12
            xpipe(1, 0, 0)
            conv_group(0, 1, 2, 0)              # 13
            xpipe(1, 0, 1)
            xpipe(1, 0, 2)
            conv_group(0, 1, 2, 1)              # 14
            xpipe(1, 0, 3)
            conv_group(0, 1, 3, 0)              # 15
            conv_group(0, 1, 3, 1)              # 16
            # b1: w0 oc0-3 with b1w1 x-pipeline interleaved, then w1
            conv_group(1, 0, 0, 0)              # 17
            xpipe(1, 1, 0)
            conv_group(1, 0, 0, 1)              # 18
            xpipe(1, 1, 1)
            conv_group(1, 0, 1, 0)              # 19
            xpipe(1, 1, 2)
            conv_group(1, 0, 1, 1)              # 20
            xpipe(1, 1, 3)
            for oc in range(2, OCC):
                for lr in range(2):
                    conv_group(1, 0, oc, lr)    # 21-24
            for oc in range(OCC):
                for lr in range(2):
                    if oc == OCC - 1 and lr == 1:
                        # split the final group so its evac/inverse/DMA
                        # tail overlaps the preceding matmuls
                        conv_group(1, 1, oc, lr, nr=8, sub=0)
                        conv_group(1, 1, oc, lr, nr=8, sub=1)
                    else:
                        conv_group(1, 1, oc, lr)    # 25-32
    nc.compile()
    return nc


class _Runner:
    """Persistent jitted PJRT executor for the SPMD kernel (axon path)."""

    def __init__(self, nc, n_cores):
        import jax
        import numpy as np
        from jax.sharding import Mesh, PartitionSpec
        try:
            from jax.experimental.shard_map import shard_map
        except ImportError:
            from jax.shard_map import shard_map
        import concourse.mybir as mybir
        from concourse.bass2jax import (
            _bass_exec_p, install_neuronx_cc_hook, partition_id_tensor,
        )

        install_neuronx_cc_hook()
        self.jax = jax
        self.n_cores = n_cores
        partition_name = (
            nc.partition_id_tensor.name if nc.partition_id_tensor else None
        )
        in_names, out_names, out_avals, zero_outs = [], [], [], []
        for alloc in nc.m.functions[0].allocations:
            if not isinstance(alloc, mybir.MemoryLocationSet):
                continue
            name = alloc.memorylocations[0].name
            if alloc.kind == "ExternalInput":
                if name != partition_name:
                    in_names.append(name)
            elif alloc.kind == "ExternalOutput":
                out_names.append(name)
                shape = tuple(alloc.tensor_shape)
                dtype = mybir.dt.np(alloc.dtype)
                out_avals.append(jax.core.ShapedArray(shape, dtype))
                zero_outs.append(np.zeros(shape, dtype))
        self.in_names, self.out_names, self.out_avals = in_names, out_names, out_avals

        def _body(*args):
            operands = list(args)
            if partition_name is not None:
                operands.append(partition_id_tensor())
            return tuple(
                _bass_exec_p.bind(
                    *operands,
                    out_avals=tuple(out_avals),
                    in_names=tuple(in_names + out_names + ([partition_name] if partition_name else [])),
                    out_names=tuple(out_names),
                    lowering_input_output_aliases=(),
                    sim_require_finite=False,
                    sim_require_nnan=False,
                    nc=nc,
                )
            )

        devices = jax.devices()[:n_cores]
        mesh = Mesh(np.asarray(devices), ("core",))
        n_params = len(in_names)
        self.fn = jax.jit(
            shard_map(
                _body, mesh=mesh,
                in_specs=(PartitionSpec("core"),) * (n_params + len(out_names)),
                out_specs=(PartitionSpec("core"),) * len(out_names),
                check_rep=False,
            ),
            keep_unused=True,
        )
        self.sharding = jax.sharding.NamedSharding(mesh, PartitionSpec("core"))
        self._dev_zeros = [
            jax.device_put(
                np.zeros((n_cores * z.shape[0], *z.shape[1:]), z.dtype), self.sharding
            )
            for z in zero_outs
        ]

    def put_inputs(self, in_maps):
        concat = [
            np.concatenate(
                [np.asarray(in_maps[c][n]) for c in range(self.n_cores)], axis=0
            )
            for n in self.in_names
        ]
        return [self.jax.device_put(a, self.sharding) for a in concat]

    def run(self, dev_args):
        outs = self.fn(*dev_args, *self._dev_zeros)
        self.jax.block_until_ready(outs)
        return outs

    def results(self, outs):
        res = []
        for c in range(self.n_cores):
            d = {}
            for i, name in enumerate(self.out_names):
                full = np.asarray(outs[i])
                d[name] = full.reshape(self.n_cores, *self.out_avals[i].shape)[c]
            res.append(d)
        return res


def _get_runner():
    if "runner" not in _CACHE:
        nc = _build()
        _CACHE["nc"] = nc
        _CACHE["runner"] = _Runner(nc, NCORES)
    return _CACHE["runner"]


_COLPERM = np.concatenate([np.arange(PC)[np.arange(PC) % 4 == m] for m in range(4)])


def _prep_inputs(x, style, weight, fc_weight, fc_bias):
    """Host-side sharding + layout marshalling. Returns per-core input maps."""
    x = np.asarray(x, dtype=np.float32)
    style = np.asarray(style, dtype=np.float32)
    weight = np.asarray(weight, dtype=np.float32)
    fc_weight = np.asarray(fc_weight, dtype=np.float32)
    fc_bias = np.asarray(fc_bias, dtype=np.float32)

    xpad = np.zeros((B, IC, PR, PC), dtype=np.float16)
    xpad[:, :, 1:H + 1, 1:W + 1] = x
    # group columns by (c mod 4) so F(4,3) d-slices are stride-1
    xpad = np.ascontiguousarray(xpad[:, :, :, _COLPERM]).reshape(B, IC, PR * PC)
    wt_host = np.ascontiguousarray(
        weight.transpose(1, 3, 2, 0)).astype(np.float16)      # [IC, KW, DY, OC]
    fcw_host = np.ascontiguousarray(fc_weight.T).astype(np.float16)
    fcb_host = np.ascontiguousarray(fc_bias.reshape(IC, 1))

    in_maps = []
    for c in range(NCORES):
        sl = slice(c * BL, (c + 1) * BL)
        in_maps.append({
            "xp": np.ascontiguousarray(xpad[sl]),
            "wt": wt_host,
            "fcw": fcw_host,
            "st": np.ascontiguousarray(style[sl].T).astype(np.float16),
            "fcb": fcb_host,
        })
    return in_maps


def kernel(x, style, weight, fc_weight, fc_bias):
    runner = _get_runner()
    in_maps = _prep_inputs(x, style, weight, fc_weight, fc_bias)
    dev_args = runner.put_inputs(in_maps)
    outs = runner.run(dev_args)
    res = runner.results(outs)
    out = np.concatenate([res[c]["y"] for c in range(NCORES)], axis=0)
    return out.astype(np.float32)
